# revision 2
# baseline (speedup 1.0000x reference)
"""MoE FFN with Sinkhorn (OT) routing — Trainium2 Bass kernel, 8 NeuronCores.

v2 strategy (slot truncation + 3-term fp8 DoubleRow + f-split balancing):
  - Router runs on host (fp32 numpy mirror of the reference); ~0.01% of FLOPs.
  - The reference combine weights slot k of token n by pi[n, k] — column k of
    the transport plan, NOT the top-k value. After Sinkhorn's final column
    normalization every pi column sums to 1, so slot weights are <= ~2e-3 and
    only slots with pi[n,k] > TAU (1e-6) contribute measurably: ~2k of 8192
    slots. Dropped-slot error is bounded by TAU * |y|max ~ 0.1% of out scale.
  - Matmuls run in fp8-e4m3 DoubleRow mode (0.5 cycles/row, 256-deep
    contraction per instruction). Accuracy comes from a 3-term residual
    scheme: A = A1 + A2 with A1 = fp8(A), A2 = fp8(A - A1) (unscaled residual,
    exploiting fp8's dynamic range so all terms share one PSUM accumulation
    group); the product (A1+A2)(B1+B2) drops only the A2*B2 term. Measured
    end-to-end rel err ~2e-3 vs the 2e-2 gate.
  - Work = per-expert token sets (C_e tokens x 32 f-tiles of SwiGLU). Each
    core runs the SAME program of 8 chunks x 4 f-tiles; chunk slots host
    (expert, f-slice) pieces: the 3 biggest experts spread across all 8 cores
    (4 f-tiles each), the remaining 5 experts' 40 pieces round-robined. Each
    chunk emits a partial y (over its 4 f-tiles) scaled by the slot weight in
    fp16; the host scatter-adds partials into the output (no collectives).
"""

import numpy as np
import ml_dtypes

import concourse.bass as bass
import concourse.mybir as mybir
import concourse.tile as tile
from concourse.bass_utils import run_bass_kernel_spmd

# Problem constants (hardcoded per contract)
B, T, D, F, E = 2, 2048, 1024, 4096, 8
N = B * T
EPS = 0.05
N_ITERS = 20
TOP_K = 2

P = 128
NK = D // P            # 8 k-tiles (d contraction)
NJT = F // P           # 32 f-tiles per expert
JC = 4                 # f-tiles per chunk
N_CORES = 8
N_CHUNKS = 8

TAU = 1e-6             # slot weight threshold
S_X, S_W, S_H = 4.0, 256.0, 16.0
INV_GU = 1.0 / (S_X * S_W)
C_H = S_H / (S_X * S_W)

FP8 = ml_dtypes.float8_e4m3fn
_f32 = np.float32

MAX_MOV = 512          # max tokens per phase-A sub-block / phase-B token group


# ---------------------------------------------------------------- host router
def _logsumexp(a, axis):
    amax = np.max(a, axis=axis, keepdims=True)
    return np.log(np.sum(np.exp(a - amax), axis=axis, keepdims=True)) + amax


def _routing(xf, gate_W):
    logits = xf @ gate_W.T
    la = (-logits) / _f32(EPS)
    for _ in range(N_ITERS):
        la = la - _logsumexp(la, axis=1)
        la = la - _logsumexp(la, axis=0)
    pi = np.exp(la)
    top2 = np.argsort(-pi, axis=1, kind="stable")[:, :TOP_K]
    return pi.astype(_f32), top2


# ---------------------------------------------------------------- planning
def _plan(counts):
    """counts: per-expert kept-token counts. Returns (shapes, asg).
    shapes: per-core uniform chunk list of (token_capacity, n_f_tiles);
    asg[core][chunk] = (expert, f_tile_start).

    Layout (sum of J = 32 per core, every weight byte shipped once):
      - 3 biggest experts: all 8 cores x 4 f-tiles each.
      - next 4 experts: (S, 16) chunks, 2 cores x 16 f-tiles each.
      - smallest expert: all 8 cores x 4 f-tiles.
    PE-heavy chunks first, the DMA-heavy 16-f chunk last: its weight stream
    drains while earlier PE-bound chunks compute, and it leaves only a light
    PE tail after the last DMA."""
    order = np.argsort(-np.asarray(counts), kind="stable")
    c = [int(counts[e]) for e in order]
    band16_cap = max(1, max(c[3:7]))
    # order found by simulator sweep: 2nd-biggest first (moderate x, PE-bound
    # start), biggest second, the DMA-heavy 16-f chunk mid, small tail chunks
    # (light PE after the final DMA)
    shapes = ((max(c[1], 1), JC), (max(c[0], 1), JC), (band16_cap, 16),
              (max(c[2], 1), JC), (max(c[7], 1), JC))
    asg = [[None] * len(shapes) for _ in range(N_CORES)]
    for k in range(N_CORES):
        asg[k][0] = (int(order[1]), JC * k)
        asg[k][1] = (int(order[0]), JC * k)
        asg[k][2] = (int(order[3 + k // 2]), 16 * (k % 2))
        asg[k][3] = (int(order[2]), JC * k)
        asg[k][4] = (int(order[7]), JC * k)
    return shapes, asg


def _tgroups(S, cap=MAX_MOV):
    """128-aligned token groups (offset, length), each <= cap."""
    out, off = [], 0
    while S - off > cap:
        out.append((off, cap))
        off += cap
    out.append((off, S - off))
    return tuple(out)


# ---------------------------------------------------------------- device kernel
def _build_kernel(shapes):
    nc = bass.Bass(
        "TRN2", target_bir_lowering=False, debug=False, num_devices=N_CORES
    )
    f32 = mybir.dt.float32
    f16 = mybir.dt.float16
    fp8 = mybir.dt.float8e4
    DR = mybir.MatmulPerfMode.DoubleRow
    SILU = mybir.ActivationFunctionType.Silu
    COPY = mybir.ActivationFunctionType.Copy
    MULT = mybir.AluOpType.mult
    ADD = mybir.AluOpType.add

    n_chunks = len(shapes)
    nsp = [-(-S // P) for S, _ in shapes]        # 128-row tiles per chunk

    x_d = [nc.declare_dram_parameter(f"x{ci}", [P, 2, NK, S], fp8,
                                     isOutput=False)
           for ci, (S, _) in enumerate(shapes)]
    # wgu: per chunk, J/2 DMA groups of 2 f-tiles: [grp, P, ft2, slot2, 2NK, P]
    wgu_d = [nc.declare_dram_parameter(
        f"wgu{ci}", [J // 2, P, 2, 2, 2 * NK, P], fp8, isOutput=False)
        for ci, (_, J) in enumerate(shapes)]
    # wd: per chunk [P, jp, dc2, blk4, 512];
    # blocks = (Wd2[2j], Wd1[2j], Wd1[2j+1], Wd2[2j+1])
    wd_d = [nc.declare_dram_parameter(
        f"wd{ci}", [P, J // 2, 2, 4, 512], fp8, isOutput=False)
        for ci, (_, J) in enumerate(shapes)]
    wv_d = [nc.declare_dram_parameter(f"wv{ci}", [P, nsp[ci]], f32,
                                      isOutput=False)
            for ci in range(n_chunks)]
    # out: per chunk [P, n_s, dc2, 512] f16; row = s*128 + p, col = dc*512 + d
    out_d = [nc.declare_dram_parameter(f"out{ci}", [P, nsp[ci], 2, 512], f16,
                                       isOutput=True)
             for ci in range(n_chunks)]

    xs_ap = [t.ap() for t in x_d]
    wgu_ap = [t.ap() for t in wgu_d]
    wd_ap = [t.ap() for t in wd_d]
    wv_ap = [t.ap() for t in wv_d]
    out_ap = [t.ap() for t in out_d]

    with tile.TileContext(nc) as tc:
        with (
            tc.tile_pool(name="xp", bufs=1) as xp,
            tc.tile_pool(name="hp", bufs=1) as hp,
            tc.tile_pool(name="wp", bufs=1) as wp,
            tc.tile_pool(name="sp", bufs=1) as sp,
            tc.tile_pool(name="yp", bufs=1) as yp,
            tc.tile_pool(name="cp", bufs=1) as cp,
            tc.tile_pool(name="ps", bufs=8, space="PSUM") as ps,
        ):
            state = {}

            def chunk_state(ci):
                if ci in state:
                    return state[ci]
                S, J = shapes[ci]
                xs = xp.tile([P, 2, NK, S], fp8, tag=f"x{ci}", bufs=1,
                             name=f"x{ci}")
                # two k-halves so early A matmuls start after half the x;
                # chunk 0 defers the second half until after the first wgu
                # piece so the first matmul starts sooner
                nc.sync.dma_start(out=xs[:, :, 0:NK // 2, :],
                                  in_=xs_ap[ci][:, :, 0:NK // 2, :])
                x_rest = (xs[:, :, NK // 2:, :], xs_ap[ci][:, :, NK // 2:, :])
                if ci != 0:
                    nc.sync.dma_start(out=x_rest[0], in_=x_rest[1])
                    x_rest = None
                # token dim padded to 128: Ldweights (stationary) APs need
                # 128-aligned strides
                h_all = hp.tile([P, 2, J, nsp[ci] * P], fp8, tag=f"h{ci}",
                                bufs=1, name=f"h{ci}")
                st = {"xs": xs, "wv": None, "h": h_all, "wgu": {},
                      "wd": {}, "x_rest": x_rest}
                state[ci] = st
                return st

            def emit_wgu(ci, grp):
                st = chunk_state(ci)
                w_sb = wp.tile([P, 2, 2, 2 * NK, P], fp8, tag="wgu", bufs=10,
                               name=f"wgu{ci}_{grp}")
                if ci == 0 and grp == 0:
                    # halves, with the deferred x half in between: the first
                    # matmul needs only (x half, wgu f-tile 0)
                    nc.sync.dma_start(out=w_sb[:, 0:1],
                                      in_=wgu_ap[ci][grp][:, 0:1])
                    if st["x_rest"] is not None:
                        nc.sync.dma_start(out=st["x_rest"][0],
                                          in_=st["x_rest"][1])
                        st["x_rest"] = None
                    nc.sync.dma_start(out=w_sb[:, 1:2],
                                      in_=wgu_ap[ci][grp][:, 1:2])
                else:
                    nc.sync.dma_start(out=w_sb, in_=wgu_ap[ci][grp])
                st["wgu"][grp] = w_sb

            def emit_wd_piece(ci, p):
                """DMA jp-pairs [2p, 2p+2) of chunk ci's wd (1MB ring tile)."""
                st = chunk_state(ci)
                J = shapes[ci][1]
                if st["wv"] is None:
                    wv = cp.tile([P, nsp[ci]], f32, tag=f"wv{ci}", bufs=1,
                                 name=f"wv{ci}")
                    nc.sync.dma_start(out=wv, in_=wv_ap[ci])
                    st["wv"] = wv
                g0, g1 = 2 * p, min(2 * p + 2, J // 2)
                if g0 < g1 and p not in st["wd"]:
                    t = wp.tile([P, 2, 2, 4, 512], fp8, tag="wd", bufs=7,
                                name=f"wd{ci}_{p}")
                    nc.sync.dma_start(out=t[:, 0:g1 - g0],
                                      in_=wd_ap[ci][:, g0:g1])
                    st["wd"][p] = t

            def emit_A(ci, j):
                """One f-tile of phase A for chunk ci."""
                st = chunk_state(ci)
                S = shapes[ci][0]
                grp, jl = divmod(j, 2)
                if grp not in st["wgu"]:
                    emit_wgu(ci, grp)
                w_sb = st["wgu"][grp]
                xs, h_all = st["xs"], st["h"]
                for si, (boff, bs) in enumerate(_tgroups(S)):
                    pg = ps.tile([P, 512], f32, tag="ps",
                                 name=f"pg{ci}_{j}_{si}")
                    pu = ps.tile([P, 512], f32, tag="ps",
                                 name=f"pu{ci}_{j}_{si}")
                    xmain = xs[:, 1, :, boff:boff + bs]
                    xpair = xs[:, :, :, boff:boff + bs]
                    for half, pt in ((0, pg), (1, pu)):
                        k0 = half * NK
                        for t in range(NK // 2):
                            nc.tensor.matmul(
                                pt[:, :bs],
                                lhsT=w_sb[:, jl, 0,
                                          k0 + 2 * t:k0 + 2 * t + 2, :],
                                rhs=xmain[:, 2 * t:2 * t + 2, :],
                                start=(t == 0), stop=False, perf_mode=DR)
                        for k in range(NK):
                            nc.tensor.matmul(
                                pt[:, :bs],
                                lhsT=w_sb[:, jl, :, k0 + k, :],
                                rhs=xpair[:, :, k, :],
                                start=False, stop=(k == NK - 1),
                                perf_mode=DR)
                    sil = sp.tile([P, 512], f32, tag="sil", bufs=3,
                                  name=f"sil{ci}_{j}_{si}")
                    nc.scalar.activation(sil[:, :bs], pg[:, :bs], SILU,
                                         scale=INV_GU)
                    hf = sp.tile([P, 512], f32, tag="hf", bufs=3,
                                 name=f"hf{ci}_{j}_{si}")
                    nc.vector.scalar_tensor_tensor(
                        hf[:, :bs], pu[:, :bs], C_H, sil[:, :bs], MULT, MULT)
                    h1 = h_all[:, 0, j, boff:boff + bs]
                    h2 = h_all[:, 1, j, boff:boff + bs]
                    nc.scalar.activation(h1, hf[:, :bs], COPY)
                    nc.vector.scalar_tensor_tensor(h2, h1, -1.0, hf[:, :bs],
                                                   MULT, ADD)

            def emit_B(ci, tg, dc):
                """One (token-group, d-half) block of phase B for chunk ci."""
                st = chunk_state(ci)
                h_all, wv = st["h"], st["wv"]
                toff, tlen = tg
                n_s = -(-tlen // P)
                py = [ps.tile([P, 512], f32, tag="ps",
                              name=f"py{ci}_{toff}_{dc}_{s}")
                      for s in range(n_s)]
                J = shapes[ci][1]
                first = True
                for jp in range(J // 2):
                    wd_sb = st["wd"][jp // 2]
                    jl = jp % 2
                    r_cross0 = wd_sb[:, jl, dc, 0:2, :]
                    r_main = wd_sb[:, jl, dc, 1:3, :]
                    r_cross1 = wd_sb[:, jl, dc, 3:1:-1, :]
                    last_jp = jp == J // 2 - 1
                    for s in range(n_s):
                        t0 = toff + s * P
                        rem = min(P, toff + tlen - t0)
                        tsl = slice(t0, t0 + rem)
                        o = py[s][:rem, :]
                        nc.tensor.matmul(
                            o, lhsT=h_all[:, :, 2 * jp, tsl], rhs=r_cross0,
                            start=first, stop=False, perf_mode=DR)
                        nc.tensor.matmul(
                            o, lhsT=h_all[:, 0, 2 * jp:2 * jp + 2, tsl],
                            rhs=r_main, start=False, stop=False, perf_mode=DR)
                        nc.tensor.matmul(
                            o, lhsT=h_all[:, :, 2 * jp + 1, tsl],
                            rhs=r_cross1,
                            start=False, stop=(last_jp and s == n_s - 1),
                            perf_mode=DR)
                    first = False
                # evict: scale by combine weight, stage fp16, one DMA out
                ysb = yp.tile([P, 4, 512], f16, tag="y", bufs=3,
                              name=f"y{ci}_{toff}_{dc}")
                for s in range(n_s):
                    t0 = toff + s * P
                    rem = min(P, toff + tlen - t0)
                    col = t0 // P
                    if s % 2 == 0:
                        nc.scalar.activation(
                            ysb[:rem, s, :], py[s][:rem, :], COPY,
                            scale=wv[:rem, col:col + 1])
                    else:
                        nc.vector.tensor_scalar_mul(
                            ysb[:rem, s, :], py[s][:rem, :],
                            wv[:rem, col:col + 1])
                # deferred by one block (see pending_out): by emission time
                # the eviction is done, so the out-DMA never head-of-line
                # blocks the SP weight stream
                s0 = toff // P
                n_full = tlen // P
                rem = tlen - n_full * P
                if n_full:
                    pending_out.append(
                        (out_ap[ci][:, s0:s0 + n_full, dc, :],
                         ysb[:, :n_full, :]))
                if rem:
                    pending_out.append(
                        (out_ap[ci][:rem, s0 + n_full, dc, :],
                         ysb[:rem, n_full, :]))

            # ---------------- emission schedule ----------------
            a_done = [0] * n_chunks
            pending_out = []
            out_hist = []

            def flush_out():
                for lst in out_hist:
                    for o, i in lst:
                        nc.sync.dma_start(out=o, in_=i)
                out_hist.clear()
                while pending_out:
                    o, i = pending_out.pop(0)
                    nc.sync.dma_start(out=o, in_=i)

            def emit_A_next(ci):
                """Emit the next phase-A f-tile of chunk ci; wd pieces ride
                along every 4th unit so B never waits on a cold wd stream."""
                j = a_done[ci]
                if j >= shapes[ci][1]:
                    return False
                emit_A(ci, j)
                if j % 4 == 3 or j == shapes[ci][1] - 1:
                    emit_wd_piece(ci, j // 4)
                a_done[ci] += 1
                return True

            def b_blocks(ci):
                # 256-token groups: 2 PSUM banks per block, so successive
                # blocks + an A-filler never exceed the 8-bank ring
                return [(ci, tg, dc) for tg in _tgroups(shapes[ci][0], 256)
                        for dc in (0, 1)]

            for ci in range(n_chunks):
                while emit_A_next(ci):
                    pass
                if ci + 1 < n_chunks:
                    chunk_state(ci + 1)   # x of next chunk ahead in queue
                if ci == n_chunks - 2:
                    # tail: drain last chunk's A, then interleave both Bs so
                    # evictions overlap the other chunk's matmuls
                    while emit_A_next(ci + 1):
                        pass
                    lists = [b_blocks(ci), b_blocks(ci + 1)]
                    inter = []
                    for i in range(max(len(l) for l in lists)):
                        for l in lists:
                            if i < len(l):
                                inter.append(l[i])
                    for bci, tg, dc in inter:
                        out_hist.append(list(pending_out))
                        pending_out.clear()
                        emit_B(bci, tg, dc)
                        if len(out_hist) >= 2:
                            for o, i in out_hist.pop(0):
                                nc.sync.dma_start(out=o, in_=i)
                    flush_out()
                    break
                blocks = b_blocks(ci)
                for bi, (bci, tg, dc) in enumerate(blocks):
                    out_hist.append(list(pending_out))
                    pending_out.clear()
                    emit_B(bci, tg, dc)
                    if ci + 1 < n_chunks:
                        emit_A_next(ci + 1)
                    if len(out_hist) >= 2:
                        for o, i in out_hist.pop(0):
                            nc.sync.dma_start(out=o, in_=i)

    _split_multiwait_instructions(nc)
    return nc


def _split_multiwait_instructions(nc, max_waits: int = 1) -> int:
    """This walrus build rejects >2 sync waits per TPB_CTRL instruction (the
    TileContext tail Drain accumulates one wait per live semaphore). Move
    excess waits onto preceding single-wait EventSemaphore instructions on the
    same engine — same-engine program order preserves the semantics."""
    n_split = 0
    for f in nc.m.functions:
        for bb in f.blocks:
            new_insts = []
            for inst in bb.instructions:
                si = inst.sync_info
                if si is not None and si.on_wait and len(si.on_wait) > max_waits:
                    waits = list(si.on_wait)
                    extra, keep = waits[:-max_waits], waits[-max_waits:]
                    for i, w in enumerate(extra):
                        new_insts.append(
                            mybir.InstEventSemaphore(
                                name=f"{inst.name}-wsplit{i}",
                                opcode="EventSemaphore",
                                engine=inst.engine,
                                sync_info=mybir.SyncInfo(on_wait=[w],
                                                         on_update=[]),
                            )
                        )
                        n_split += 1
                    inst.sync_info = mybir.SyncInfo(
                        on_wait=keep, on_update=list(si.on_update or [])
                    )
                new_insts.append(inst)
            bb.instructions[:] = new_insts
    return n_split


# ---------------------------------------------------------------- host packing
def _fp8_pair(a):
    """Split fp32 array into (main, residual) e4m3 pair; a ~= main + residual."""
    a1 = a.astype(FP8)
    a2 = (a - a1.astype(np.float32)).astype(FP8)
    return a1, a2


def _pack_x(xe_pair, S):
    """xe_pair: (x1, x2) arrays [C, D]. Returns [P, 2, NK, S] fp8."""
    x1, x2 = xe_pair
    C = x1.shape[0]
    outp = np.zeros((P, 2, NK, S), dtype=FP8)
    # [C, D] -> [P, NK, C]
    outp[:, 1, :, :C] = x1.reshape(C, NK, P).transpose(2, 1, 0)
    outp[:, 0, :, :C] = x2.reshape(C, NK, P).transpose(2, 1, 0)
    return outp


def _prep_weights(W_gate, W_up, W_down):
    """Precompute per-expert packed fp8 weight pair tensors.
    WGU8: [E, 32ft, P, 2slot, 2NK, P]; WD8: [E, 32ft, P, 2slot, 2dc, 512]."""
    WGU8 = np.empty((E, NJT, P, 2, 2 * NK, P), dtype=FP8)
    WD8 = np.empty((E, NJT, P, 2, 2, 512), dtype=FP8)
    for e in range(E):
        g1, g2 = _fp8_pair(W_gate[e] * S_W)     # [F, D]
        u1, u2 = _fp8_pair(W_up[e] * S_W)
        d1, d2 = _fp8_pair(W_down[e] * S_W)     # [D, F]
        for slot, (gg, uu) in ((0, (g1, u1)), (1, (g2, u2))):
            # [F, D] -> [ft, P(p), NK(k), P(m)]: value W[ft*128+m, k*128+p]
            gt = gg.reshape(NJT, P, NK, P).transpose(0, 3, 2, 1)
            ut = uu.reshape(NJT, P, NK, P).transpose(0, 3, 2, 1)
            WGU8[e, :, :, slot, :NK, :] = gt
            WGU8[e, :, :, slot, NK:, :] = ut
        for slot, dd in ((0, d1), (1, d2)):
            # [D, F] -> [ft, P(p), dc, 512(d)]: value Wd[dc*512+d, ft*128+p]
            WD8[e, :, :, slot, :, :] = dd.reshape(2, 512, NJT, P).transpose(
                2, 3, 0, 1)
    return WGU8, WD8


def _pack_wgu(WGU8, e, f0, J):
    """-> [J//2 grp, P, 2ft, 2slot, 2NK, P] for f-tiles f0..f0+J."""
    blk = WGU8[e, f0:f0 + J]                     # [J, P, 2, 16, P]
    return np.ascontiguousarray(
        blk.reshape(J // 2, 2, P, 2, 2 * NK, P).transpose(0, 2, 1, 3, 4, 5))


def _pack_wd(WD8, e, f0, J):
    """-> [P, J//2 jp, 2dc, 4blk, 512] with blocks
    (Wd2[2j], Wd1[2j], Wd1[2j+1], Wd2[2j+1])."""
    outp = np.empty((P, J // 2, 2, 4, 512), dtype=FP8)
    for jp in range(J // 2):
        ja, jb = f0 + 2 * jp, f0 + 2 * jp + 1
        for dc in range(2):
            outp[:, jp, dc, 0, :] = WD8[e, ja, :, 1, dc, :]
            outp[:, jp, dc, 1, :] = WD8[e, ja, :, 0, dc, :]
            outp[:, jp, dc, 2, :] = WD8[e, jb, :, 0, dc, :]
            outp[:, jp, dc, 3, :] = WD8[e, jb, :, 1, dc, :]
    return outp


_BUILT = {}


def _get_kernel(shapes):
    if shapes not in _BUILT:
        _BUILT[shapes] = _build_kernel(shapes)
    return _BUILT[shapes]


def kernel(x, gate_W, W_gate, W_up, W_down, _return_results=False,
           _run_kwargs=None):
    x = np.asarray(x, dtype=_f32)
    gate_W = np.asarray(gate_W, dtype=_f32)
    W_gate = np.asarray(W_gate, dtype=_f32)
    W_up = np.asarray(W_up, dtype=_f32)
    W_down = np.asarray(W_down, dtype=_f32)
    xf = np.ascontiguousarray(x.reshape(N, D))
    pi, top2 = _routing(xf, gate_W)

    # kept slots per expert
    toks, wts = [], []
    for e in range(E):
        tl, wl = [], []
        for k in range(TOP_K):
            m = (top2[:, k] == e) & (pi[:, k] > TAU)
            tl.append(np.nonzero(m)[0])
            wl.append(pi[m, k])
        toks.append(np.concatenate(tl))
        wts.append(np.concatenate(wl).astype(_f32))
    counts = [len(t) for t in toks]

    shapes, asg = _plan(counts)
    n_chunks = len(shapes)
    nsp = [-(-S // P) for S, _ in shapes]

    WGU8, WD8 = _prep_weights(W_gate, W_up, W_down)
    # per-expert packed x (shared across cores/chunks of the same expert)
    x_pair = {e: _fp8_pair(xf[toks[e]] * S_X) for e in range(E)
              if counts[e] > 0}
    x_packed = {}

    def x_for(e, S):
        key = (e, S)
        if key not in x_packed:
            x_packed[key] = _pack_x(x_pair[e], S)
        return x_packed[key]

    def wv_for(e, S):
        ns = -(-S // P)
        w = np.zeros(ns * P, dtype=_f32)
        w[:counts[e]] = wts[e] / _f32(S_H * S_W)
        return np.ascontiguousarray(w.reshape(ns, P).T)

    in_maps = []
    for k in range(N_CORES):
        m = {}
        for ci in range(n_chunks):
            e, f0 = asg[k][ci]
            S, J = shapes[ci]
            if counts[e] > 0:
                m[f"x{ci}"] = x_for(e, S)
                m[f"wv{ci}"] = wv_for(e, S)
            else:
                m[f"x{ci}"] = np.zeros((P, 2, NK, S), dtype=FP8)
                m[f"wv{ci}"] = np.zeros((P, nsp[ci]), dtype=_f32)
            m[f"wgu{ci}"] = _pack_wgu(WGU8, e, f0, J)
            m[f"wd{ci}"] = _pack_wd(WD8, e, f0, J)
        in_maps.append(m)

    nc = _get_kernel(shapes)
    res = run_bass_kernel_spmd(
        nc, in_maps, list(range(N_CORES)), **(_run_kwargs or {})
    )

    out_full = np.zeros((N, D), dtype=_f32)
    for k in range(N_CORES):
        for ci in range(n_chunks):
            e, f0 = asg[k][ci]
            if counts[e] == 0:
                continue
            arr = res.results[k][f"out{ci}"]        # [P, n_s, 2, 512] f16
            rows = arr.transpose(1, 0, 2, 3).reshape(nsp[ci] * P, D)
            out_full[toks[e]] += rows[:counts[e]].astype(_f32)

    out_full = out_full.reshape(B, T, D)
    if _return_results:
        return out_full, res
    return out_full


# revision 3
# speedup vs baseline: 1.0242x; 1.0242x over previous
"""MoE FFN with Sinkhorn (OT) routing — Trainium2 Bass kernel, 8 NeuronCores.

v2 strategy (slot truncation + 3-term fp8 DoubleRow + f-split balancing):
  - Router runs on host (fp32 numpy mirror of the reference); ~0.01% of FLOPs.
  - The reference combine weights slot k of token n by pi[n, k] — column k of
    the transport plan, NOT the top-k value. After Sinkhorn's final column
    normalization every pi column sums to 1, so slot weights are <= ~2e-3 and
    only slots with pi[n,k] > TAU (1e-6) contribute measurably: ~2k of 8192
    slots. Dropped-slot error is bounded by TAU * |y|max ~ 0.1% of out scale.
  - Matmuls run in fp8-e4m3 DoubleRow mode (0.5 cycles/row, 256-deep
    contraction per instruction). Accuracy comes from a 3-term residual
    scheme: A = A1 + A2 with A1 = fp8(A), A2 = fp8(A - A1) (unscaled residual,
    exploiting fp8's dynamic range so all terms share one PSUM accumulation
    group); the product (A1+A2)(B1+B2) drops only the A2*B2 term. Measured
    end-to-end rel err ~2e-3 vs the 2e-2 gate.
  - Work = per-expert token sets (C_e tokens x 32 f-tiles of SwiGLU). Each
    core runs the SAME program of 8 chunks x 4 f-tiles; chunk slots host
    (expert, f-slice) pieces: the 3 biggest experts spread across all 8 cores
    (4 f-tiles each), the remaining 5 experts' 40 pieces round-robined. Each
    chunk emits a partial y (over its 4 f-tiles) scaled by the slot weight in
    fp16; the host scatter-adds partials into the output (no collectives).
"""

import numpy as np
import ml_dtypes

import concourse.bass as bass
import concourse.mybir as mybir
import concourse.tile as tile
from concourse.bass_utils import run_bass_kernel_spmd

# Problem constants (hardcoded per contract)
B, T, D, F, E = 2, 2048, 1024, 4096, 8
N = B * T
EPS = 0.05
N_ITERS = 20
TOP_K = 2

P = 128
NK = D // P            # 8 k-tiles (d contraction)
NJT = F // P           # 32 f-tiles per expert
JC = 4                 # f-tiles per chunk
N_CORES = 8
N_CHUNKS = 8

TAU = 1e-6             # slot weight threshold
S_X, S_W, S_H = 4.0, 256.0, 16.0
INV_GU = 1.0 / (S_X * S_W)
C_H = S_H / (S_X * S_W)

FP8 = ml_dtypes.float8_e4m3fn
_f32 = np.float32

MAX_MOV = 512          # max tokens per phase-A sub-block / phase-B token group


# ---------------------------------------------------------------- host router
def _logsumexp(a, axis):
    amax = np.max(a, axis=axis, keepdims=True)
    return np.log(np.sum(np.exp(a - amax), axis=axis, keepdims=True)) + amax


def _routing(xf, gate_W):
    logits = xf @ gate_W.T
    la = (-logits) / _f32(EPS)
    for _ in range(N_ITERS):
        la = la - _logsumexp(la, axis=1)
        la = la - _logsumexp(la, axis=0)
    pi = np.exp(la)
    top2 = np.argsort(-pi, axis=1, kind="stable")[:, :TOP_K]
    return pi.astype(_f32), top2


# ---------------------------------------------------------------- planning
def _plan(counts):
    """counts: per-expert kept-token counts. Returns (shapes, asg).
    shapes: per-core uniform chunk list of (token_capacity, n_f_tiles);
    asg[core][chunk] = (expert, f_tile_start).

    Layout (sum of J = 32 per core, every weight byte shipped once):
      - 3 biggest experts: all 8 cores x 4 f-tiles each.
      - next 4 experts: (S, 16) chunks, 2 cores x 16 f-tiles each.
      - smallest expert: all 8 cores x 4 f-tiles.
    PE-heavy chunks first, the DMA-heavy 16-f chunk last: its weight stream
    drains while earlier PE-bound chunks compute, and it leaves only a light
    PE tail after the last DMA."""
    order = np.argsort(-np.asarray(counts), kind="stable")
    c = [int(counts[e]) for e in order]
    band16_cap = max(1, max(c[3:7]))
    # order found by simulator sweep: 2nd-biggest first (moderate x, PE-bound
    # start), biggest second, the DMA-heavy 16-f chunk mid, small tail chunks
    # (light PE after the final DMA)
    shapes = ((max(c[1], 1), JC), (max(c[0], 1), JC), (band16_cap, 16),
              (max(c[2], 1), JC), (max(c[7], 1), JC))
    asg = [[None] * len(shapes) for _ in range(N_CORES)]
    for k in range(N_CORES):
        asg[k][0] = (int(order[1]), JC * k)
        asg[k][1] = (int(order[0]), JC * k)
        asg[k][2] = (int(order[3 + k // 2]), 16 * (k % 2))
        asg[k][3] = (int(order[2]), JC * k)
        asg[k][4] = (int(order[7]), JC * k)
    return shapes, asg


def _tgroups(S, cap=MAX_MOV):
    """128-aligned token groups (offset, length), each <= cap."""
    out, off = [], 0
    while S - off > cap:
        out.append((off, cap))
        off += cap
    out.append((off, S - off))
    return tuple(out)


# ---------------------------------------------------------------- device kernel
def _build_kernel(shapes):
    nc = bass.Bass(
        "TRN2", target_bir_lowering=False, debug=False, num_devices=N_CORES
    )
    f32 = mybir.dt.float32
    f16 = mybir.dt.float16
    fp8 = mybir.dt.float8e4
    DR = mybir.MatmulPerfMode.DoubleRow
    SILU = mybir.ActivationFunctionType.Silu
    COPY = mybir.ActivationFunctionType.Copy
    MULT = mybir.AluOpType.mult
    ADD = mybir.AluOpType.add

    n_chunks = len(shapes)
    nsp = [-(-S // P) for S, _ in shapes]        # 128-row tiles per chunk

    x_d = [nc.declare_dram_parameter(f"x{ci}", [P, 2, NK, S], fp8,
                                     isOutput=False)
           for ci, (S, _) in enumerate(shapes)]
    # wgu: per chunk, J/2 DMA groups of 2 f-tiles: [grp, P, ft2, slot2, 2NK, P]
    wgu_d = [nc.declare_dram_parameter(
        f"wgu{ci}", [J // 2, P, 2, 2, 2 * NK, P], fp8, isOutput=False)
        for ci, (_, J) in enumerate(shapes)]
    # wd: per chunk [P, jp, dc2, blk4, 512];
    # blocks = (Wd2[2j], Wd1[2j], Wd1[2j+1], Wd2[2j+1])
    wd_d = [nc.declare_dram_parameter(
        f"wd{ci}", [P, J // 2, 2, 4, 512], fp8, isOutput=False)
        for ci, (_, J) in enumerate(shapes)]
    wv_d = [nc.declare_dram_parameter(f"wv{ci}", [P, nsp[ci]], f32,
                                      isOutput=False)
            for ci in range(n_chunks)]
    # out: per chunk [P, n_s, dc2, 512] f16; row = s*128 + p, col = dc*512 + d
    out_d = [nc.declare_dram_parameter(f"out{ci}", [P, nsp[ci], 2, 512], f16,
                                       isOutput=True)
             for ci in range(n_chunks)]

    xs_ap = [t.ap() for t in x_d]
    wgu_ap = [t.ap() for t in wgu_d]
    wd_ap = [t.ap() for t in wd_d]
    wv_ap = [t.ap() for t in wv_d]
    out_ap = [t.ap() for t in out_d]

    with tile.TileContext(nc) as tc:
        with (
            tc.tile_pool(name="xp", bufs=1) as xp,
            tc.tile_pool(name="hp", bufs=1) as hp,
            tc.tile_pool(name="wp", bufs=1) as wp,
            tc.tile_pool(name="sp", bufs=1) as sp,
            tc.tile_pool(name="yp", bufs=1) as yp,
            tc.tile_pool(name="cp", bufs=1) as cp,
            tc.tile_pool(name="ps", bufs=8, space="PSUM") as ps,
        ):
            state = {}

            def chunk_state(ci):
                if ci in state:
                    return state[ci]
                S, J = shapes[ci]
                xs = xp.tile([P, 2, NK, S], fp8, tag=f"x{ci}", bufs=1,
                             name=f"x{ci}")
                # two k-halves so early A matmuls start after half the x;
                # chunk 0 defers the second half until after the first wgu
                # piece so the first matmul starts sooner
                nc.sync.dma_start(out=xs[:, :, 0:NK // 2, :],
                                  in_=xs_ap[ci][:, :, 0:NK // 2, :])
                x_rest = (xs[:, :, NK // 2:, :], xs_ap[ci][:, :, NK // 2:, :])
                if ci != 0:
                    nc.sync.dma_start(out=x_rest[0], in_=x_rest[1])
                    x_rest = None
                # token dim padded to 128: Ldweights (stationary) APs need
                # 128-aligned strides
                h_all = hp.tile([P, 2, J, nsp[ci] * P], fp8, tag=f"h{ci}",
                                bufs=1, name=f"h{ci}")
                st = {"xs": xs, "wv": None, "h": h_all, "wgu": {},
                      "wd": {}, "x_rest": x_rest}
                state[ci] = st
                return st

            def emit_wgu(ci, grp):
                st = chunk_state(ci)
                w_sb = wp.tile([P, 2, 2, 2 * NK, P], fp8, tag="wgu", bufs=10,
                               name=f"wgu{ci}_{grp}")
                if ci == 0 and grp == 0:
                    # halves, with the deferred x half in between: the first
                    # matmul needs only (x half, wgu f-tile 0)
                    nc.sync.dma_start(out=w_sb[:, 0:1],
                                      in_=wgu_ap[ci][grp][:, 0:1])
                    if st["x_rest"] is not None:
                        nc.sync.dma_start(out=st["x_rest"][0],
                                          in_=st["x_rest"][1])
                        st["x_rest"] = None
                    nc.sync.dma_start(out=w_sb[:, 1:2],
                                      in_=wgu_ap[ci][grp][:, 1:2])
                else:
                    nc.sync.dma_start(out=w_sb, in_=wgu_ap[ci][grp])
                st["wgu"][grp] = w_sb

            def emit_wd_piece(ci, p):
                """DMA jp-pairs [2p, 2p+2) of chunk ci's wd (1MB ring tile)."""
                st = chunk_state(ci)
                J = shapes[ci][1]
                if st["wv"] is None:
                    wv = cp.tile([P, nsp[ci]], f32, tag=f"wv{ci}", bufs=1,
                                 name=f"wv{ci}")
                    nc.sync.dma_start(out=wv, in_=wv_ap[ci])
                    st["wv"] = wv
                g0, g1 = 2 * p, min(2 * p + 2, J // 2)
                if g0 < g1 and p not in st["wd"]:
                    t = wp.tile([P, 2, 2, 4, 512], fp8, tag="wd", bufs=7,
                                name=f"wd{ci}_{p}")
                    nc.sync.dma_start(out=t[:, 0:g1 - g0],
                                      in_=wd_ap[ci][:, g0:g1])
                    st["wd"][p] = t

            def emit_A(ci, j):
                """One f-tile of phase A for chunk ci."""
                st = chunk_state(ci)
                S = shapes[ci][0]
                grp, jl = divmod(j, 2)
                if grp not in st["wgu"]:
                    emit_wgu(ci, grp)
                w_sb = st["wgu"][grp]
                xs, h_all = st["xs"], st["h"]
                for si, (boff, bs) in enumerate(_tgroups(S)):
                    pg = ps.tile([P, 512], f32, tag="ps",
                                 name=f"pg{ci}_{j}_{si}")
                    pu = ps.tile([P, 512], f32, tag="ps",
                                 name=f"pu{ci}_{j}_{si}")
                    xmain = xs[:, 1, :, boff:boff + bs]
                    xpair = xs[:, :, :, boff:boff + bs]
                    for half, pt in ((0, pg), (1, pu)):
                        k0 = half * NK
                        for t in range(NK // 2):
                            nc.tensor.matmul(
                                pt[:, :bs],
                                lhsT=w_sb[:, jl, 0,
                                          k0 + 2 * t:k0 + 2 * t + 2, :],
                                rhs=xmain[:, 2 * t:2 * t + 2, :],
                                start=(t == 0), stop=False, perf_mode=DR)
                        for k in range(NK):
                            nc.tensor.matmul(
                                pt[:, :bs],
                                lhsT=w_sb[:, jl, :, k0 + k, :],
                                rhs=xpair[:, :, k, :],
                                start=False, stop=(k == NK - 1),
                                perf_mode=DR)
                    sil = sp.tile([P, 512], f32, tag="sil", bufs=3,
                                  name=f"sil{ci}_{j}_{si}")
                    nc.scalar.activation(sil[:, :bs], pg[:, :bs], SILU,
                                         scale=INV_GU)
                    hf = sp.tile([P, 512], f32, tag="hf", bufs=3,
                                 name=f"hf{ci}_{j}_{si}")
                    nc.vector.scalar_tensor_tensor(
                        hf[:, :bs], pu[:, :bs], C_H, sil[:, :bs], MULT, MULT)
                    h1 = h_all[:, 0, j, boff:boff + bs]
                    h2 = h_all[:, 1, j, boff:boff + bs]
                    nc.scalar.activation(h1, hf[:, :bs], COPY)
                    nc.vector.scalar_tensor_tensor(h2, h1, -1.0, hf[:, :bs],
                                                   MULT, ADD)

            def emit_B(ci, tg, dc):
                """One (token-group, d-half) block of phase B for chunk ci."""
                st = chunk_state(ci)
                h_all, wv = st["h"], st["wv"]
                toff, tlen = tg
                n_s = -(-tlen // P)
                py = [ps.tile([P, 512], f32, tag="ps",
                              name=f"py{ci}_{toff}_{dc}_{s}")
                      for s in range(n_s)]
                J = shapes[ci][1]
                first = True
                for jp in range(J // 2):
                    wd_sb = st["wd"][jp // 2]
                    jl = jp % 2
                    r_cross0 = wd_sb[:, jl, dc, 0:2, :]
                    r_main = wd_sb[:, jl, dc, 1:3, :]
                    r_cross1 = wd_sb[:, jl, dc, 3:1:-1, :]
                    last_jp = jp == J // 2 - 1
                    for s in range(n_s):
                        t0 = toff + s * P
                        rem = min(P, toff + tlen - t0)
                        tsl = slice(t0, t0 + rem)
                        o = py[s][:rem, :]
                        nc.tensor.matmul(
                            o, lhsT=h_all[:, :, 2 * jp, tsl], rhs=r_cross0,
                            start=first, stop=False, perf_mode=DR)
                        nc.tensor.matmul(
                            o, lhsT=h_all[:, 0, 2 * jp:2 * jp + 2, tsl],
                            rhs=r_main, start=False, stop=False, perf_mode=DR)
                        nc.tensor.matmul(
                            o, lhsT=h_all[:, :, 2 * jp + 1, tsl],
                            rhs=r_cross1,
                            start=False, stop=(last_jp and s == n_s - 1),
                            perf_mode=DR)
                    first = False
                # evict: scale by combine weight, stage fp16, one DMA out
                ysb = yp.tile([P, 4, 512], f16, tag="y", bufs=3,
                              name=f"y{ci}_{toff}_{dc}")
                for s in range(n_s):
                    t0 = toff + s * P
                    rem = min(P, toff + tlen - t0)
                    col = t0 // P
                    if s % 2 == 0:
                        nc.scalar.activation(
                            ysb[:rem, s, :], py[s][:rem, :], COPY,
                            scale=wv[:rem, col:col + 1])
                    else:
                        nc.vector.tensor_scalar_mul(
                            ysb[:rem, s, :], py[s][:rem, :],
                            wv[:rem, col:col + 1])
                # deferred by one block (see pending_out): by emission time
                # the eviction is done, so the out-DMA never head-of-line
                # blocks the SP weight stream
                s0 = toff // P
                n_full = tlen // P
                rem = tlen - n_full * P
                if n_full:
                    pending_out.append(
                        (out_ap[ci][:, s0:s0 + n_full, dc, :],
                         ysb[:, :n_full, :]))
                if rem:
                    pending_out.append(
                        (out_ap[ci][:rem, s0 + n_full, dc, :],
                         ysb[:rem, n_full, :]))

            # ---------------- emission schedule ----------------
            a_done = [0] * n_chunks
            pending_out = []
            out_hist = []

            def flush_out():
                for lst in out_hist:
                    for o, i in lst:
                        nc.sync.dma_start(out=o, in_=i)
                out_hist.clear()
                while pending_out:
                    o, i = pending_out.pop(0)
                    nc.sync.dma_start(out=o, in_=i)

            def emit_A_next(ci):
                """Emit the next phase-A f-tile of chunk ci; wd pieces ride
                along every 4th unit so B never waits on a cold wd stream."""
                j = a_done[ci]
                if j >= shapes[ci][1]:
                    return False
                emit_A(ci, j)
                if j % 4 == 3 or j == shapes[ci][1] - 1:
                    emit_wd_piece(ci, j // 4)
                a_done[ci] += 1
                return True

            def b_blocks(ci):
                # 256-token groups: 2 PSUM banks per block, so successive
                # blocks + an A-filler never exceed the 8-bank ring
                return [(ci, tg, dc) for tg in _tgroups(shapes[ci][0], 256)
                        for dc in (0, 1)]

            for ci in range(n_chunks):
                while emit_A_next(ci):
                    pass
                if ci + 1 < n_chunks:
                    chunk_state(ci + 1)   # x of next chunk ahead in queue
                if ci == n_chunks - 2:
                    # tail: drain last chunk's A, then interleave both Bs so
                    # evictions overlap the other chunk's matmuls
                    while emit_A_next(ci + 1):
                        pass
                    lists = [b_blocks(ci), b_blocks(ci + 1)]
                    inter = []
                    for i in range(max(len(l) for l in lists)):
                        for l in lists:
                            if i < len(l):
                                inter.append(l[i])
                    for bci, tg, dc in inter:
                        out_hist.append(list(pending_out))
                        pending_out.clear()
                        emit_B(bci, tg, dc)
                        if len(out_hist) >= 2:
                            for o, i in out_hist.pop(0):
                                nc.sync.dma_start(out=o, in_=i)
                    flush_out()
                    break
                blocks = b_blocks(ci)
                for bi, (bci, tg, dc) in enumerate(blocks):
                    out_hist.append(list(pending_out))
                    pending_out.clear()
                    emit_B(bci, tg, dc)
                    if ci + 1 < n_chunks:
                        # adaptive pacing: finish ALL of the next chunk's A
                        # by the end of this B phase, so a DMA-heavy next
                        # chunk's stream interleaves with our PE-dense blocks
                        rem_gaps = len(blocks) - bi
                        rem_units = shapes[ci + 1][1] - a_done[ci + 1]
                        n_fill = -(-rem_units // max(1, rem_gaps))
                        for _ in range(n_fill):
                            emit_A_next(ci + 1)
                    if len(out_hist) >= 2:
                        for o, i in out_hist.pop(0):
                            nc.sync.dma_start(out=o, in_=i)

    _split_multiwait_instructions(nc)
    return nc


def _split_multiwait_instructions(nc, max_waits: int = 1) -> int:
    """This walrus build rejects >2 sync waits per TPB_CTRL instruction (the
    TileContext tail Drain accumulates one wait per live semaphore). Move
    excess waits onto preceding single-wait EventSemaphore instructions on the
    same engine — same-engine program order preserves the semantics."""
    n_split = 0
    for f in nc.m.functions:
        for bb in f.blocks:
            new_insts = []
            for inst in bb.instructions:
                si = inst.sync_info
                if si is not None and si.on_wait and len(si.on_wait) > max_waits:
                    waits = list(si.on_wait)
                    extra, keep = waits[:-max_waits], waits[-max_waits:]
                    for i, w in enumerate(extra):
                        new_insts.append(
                            mybir.InstEventSemaphore(
                                name=f"{inst.name}-wsplit{i}",
                                opcode="EventSemaphore",
                                engine=inst.engine,
                                sync_info=mybir.SyncInfo(on_wait=[w],
                                                         on_update=[]),
                            )
                        )
                        n_split += 1
                    inst.sync_info = mybir.SyncInfo(
                        on_wait=keep, on_update=list(si.on_update or [])
                    )
                new_insts.append(inst)
            bb.instructions[:] = new_insts
    return n_split


# ---------------------------------------------------------------- host packing
def _fp8_pair(a):
    """Split fp32 array into (main, residual) e4m3 pair; a ~= main + residual."""
    a1 = a.astype(FP8)
    a2 = (a - a1.astype(np.float32)).astype(FP8)
    return a1, a2


def _pack_x(xe_pair, S):
    """xe_pair: (x1, x2) arrays [C, D]. Returns [P, 2, NK, S] fp8."""
    x1, x2 = xe_pair
    C = x1.shape[0]
    outp = np.zeros((P, 2, NK, S), dtype=FP8)
    # [C, D] -> [P, NK, C]
    outp[:, 1, :, :C] = x1.reshape(C, NK, P).transpose(2, 1, 0)
    outp[:, 0, :, :C] = x2.reshape(C, NK, P).transpose(2, 1, 0)
    return outp


def _prep_weights(W_gate, W_up, W_down):
    """Precompute per-expert packed fp8 weight pair tensors.
    WGU8: [E, 32ft, P, 2slot, 2NK, P]; WD8: [E, 32ft, P, 2slot, 2dc, 512]."""
    WGU8 = np.empty((E, NJT, P, 2, 2 * NK, P), dtype=FP8)
    WD8 = np.empty((E, NJT, P, 2, 2, 512), dtype=FP8)
    for e in range(E):
        g1, g2 = _fp8_pair(W_gate[e] * S_W)     # [F, D]
        u1, u2 = _fp8_pair(W_up[e] * S_W)
        d1, d2 = _fp8_pair(W_down[e] * S_W)     # [D, F]
        for slot, (gg, uu) in ((0, (g1, u1)), (1, (g2, u2))):
            # [F, D] -> [ft, P(p), NK(k), P(m)]: value W[ft*128+m, k*128+p]
            gt = gg.reshape(NJT, P, NK, P).transpose(0, 3, 2, 1)
            ut = uu.reshape(NJT, P, NK, P).transpose(0, 3, 2, 1)
            WGU8[e, :, :, slot, :NK, :] = gt
            WGU8[e, :, :, slot, NK:, :] = ut
        for slot, dd in ((0, d1), (1, d2)):
            # [D, F] -> [ft, P(p), dc, 512(d)]: value Wd[dc*512+d, ft*128+p]
            WD8[e, :, :, slot, :, :] = dd.reshape(2, 512, NJT, P).transpose(
                2, 3, 0, 1)
    return WGU8, WD8


def _pack_wgu(WGU8, e, f0, J):
    """-> [J//2 grp, P, 2ft, 2slot, 2NK, P] for f-tiles f0..f0+J."""
    blk = WGU8[e, f0:f0 + J]                     # [J, P, 2, 16, P]
    return np.ascontiguousarray(
        blk.reshape(J // 2, 2, P, 2, 2 * NK, P).transpose(0, 2, 1, 3, 4, 5))


def _pack_wd(WD8, e, f0, J):
    """-> [P, J//2 jp, 2dc, 4blk, 512] with blocks
    (Wd2[2j], Wd1[2j], Wd1[2j+1], Wd2[2j+1])."""
    outp = np.empty((P, J // 2, 2, 4, 512), dtype=FP8)
    for jp in range(J // 2):
        ja, jb = f0 + 2 * jp, f0 + 2 * jp + 1
        for dc in range(2):
            outp[:, jp, dc, 0, :] = WD8[e, ja, :, 1, dc, :]
            outp[:, jp, dc, 1, :] = WD8[e, ja, :, 0, dc, :]
            outp[:, jp, dc, 2, :] = WD8[e, jb, :, 0, dc, :]
            outp[:, jp, dc, 3, :] = WD8[e, jb, :, 1, dc, :]
    return outp


_BUILT = {}


def _get_kernel(shapes):
    if shapes not in _BUILT:
        _BUILT[shapes] = _build_kernel(shapes)
    return _BUILT[shapes]


def kernel(x, gate_W, W_gate, W_up, W_down, _return_results=False,
           _run_kwargs=None):
    x = np.asarray(x, dtype=_f32)
    gate_W = np.asarray(gate_W, dtype=_f32)
    W_gate = np.asarray(W_gate, dtype=_f32)
    W_up = np.asarray(W_up, dtype=_f32)
    W_down = np.asarray(W_down, dtype=_f32)
    xf = np.ascontiguousarray(x.reshape(N, D))
    pi, top2 = _routing(xf, gate_W)

    # kept slots per expert
    toks, wts = [], []
    for e in range(E):
        tl, wl = [], []
        for k in range(TOP_K):
            m = (top2[:, k] == e) & (pi[:, k] > TAU)
            tl.append(np.nonzero(m)[0])
            wl.append(pi[m, k])
        toks.append(np.concatenate(tl))
        wts.append(np.concatenate(wl).astype(_f32))
    counts = [len(t) for t in toks]

    shapes, asg = _plan(counts)
    n_chunks = len(shapes)
    nsp = [-(-S // P) for S, _ in shapes]

    WGU8, WD8 = _prep_weights(W_gate, W_up, W_down)
    # per-expert packed x (shared across cores/chunks of the same expert)
    x_pair = {e: _fp8_pair(xf[toks[e]] * S_X) for e in range(E)
              if counts[e] > 0}
    x_packed = {}

    def x_for(e, S):
        key = (e, S)
        if key not in x_packed:
            x_packed[key] = _pack_x(x_pair[e], S)
        return x_packed[key]

    def wv_for(e, S):
        ns = -(-S // P)
        w = np.zeros(ns * P, dtype=_f32)
        w[:counts[e]] = wts[e] / _f32(S_H * S_W)
        return np.ascontiguousarray(w.reshape(ns, P).T)

    in_maps = []
    for k in range(N_CORES):
        m = {}
        for ci in range(n_chunks):
            e, f0 = asg[k][ci]
            S, J = shapes[ci]
            if counts[e] > 0:
                m[f"x{ci}"] = x_for(e, S)
                m[f"wv{ci}"] = wv_for(e, S)
            else:
                m[f"x{ci}"] = np.zeros((P, 2, NK, S), dtype=FP8)
                m[f"wv{ci}"] = np.zeros((P, nsp[ci]), dtype=_f32)
            m[f"wgu{ci}"] = _pack_wgu(WGU8, e, f0, J)
            m[f"wd{ci}"] = _pack_wd(WD8, e, f0, J)
        in_maps.append(m)

    nc = _get_kernel(shapes)
    res = run_bass_kernel_spmd(
        nc, in_maps, list(range(N_CORES)), **(_run_kwargs or {})
    )

    out_full = np.zeros((N, D), dtype=_f32)
    for k in range(N_CORES):
        for ci in range(n_chunks):
            e, f0 = asg[k][ci]
            if counts[e] == 0:
                continue
            arr = res.results[k][f"out{ci}"]        # [P, n_s, 2, 512] f16
            rows = arr.transpose(1, 0, 2, 3).reshape(nsp[ci] * P, D)
            out_full[toks[e]] += rows[:counts[e]].astype(_f32)

    out_full = out_full.reshape(B, T, D)
    if _return_results:
        return out_full, res
    return out_full
